# revision 1
# baseline (speedup 1.0000x reference)
"""AttentionRNN Trainium2 Bass kernel, v2.

Strategy (vs the v1 baseline):
- 16 batch chunks of 256 rows; each of the 8 cores runs TWO independent
  LSTM scans (chunk 2c and 2c+1), each 288 steps (256 + 32 warmup steps to
  converge the contractive LSTM state; measured contamination ~2e-6).
  The two scans interleave so the PE stays busy during one scan's softmax +
  LSTM pointwise phases.
- Gates matmul uses full 128-column stationary weight loads (FWL-eligible)
  with the gates produced transposed ([gate, L] layout) so the pointwise
  work uses all 128 partitions.
- All sigmoid/tanh scales are folded into host-packed weights so ONE tanh
  over [128, 512] computes every gate nonlinearity (sigmoid(x) =
  (tanh(x/2)+1)/2; h is stored as 2h, c as 2c).
- Stage A (conv1 residual block): host packs the input transposed into a
  guarded f-major layout ([28, 2 + 34*s + l] with zero guard columns between
  samples), so the conv over l is 3 shifted matmuls; elu/mask/residual/u_a
  run on 510-column strips.
- bias_mat / unpacked_masks are all-zero / all-one per the problem spec;
  kernel() checks this at runtime and falls back to a generic build if not.
"""

import numpy as np
import ml_dtypes

import concourse.bass as bass
import concourse.mybir as mybir
import concourse.tile as tile
from concourse import bacc
from concourse.bass_utils import run_bass_kernel_spmd

dt = mybir.dt
AF = mybir.ActivationFunctionType
ALU = mybir.AluOpType

B = 4096
F = 28
L = 32
H = 512
N_CORES = 8
NCH = 2                       # scans (chunks) per core
CHUNK = B // (N_CORES * NCH)  # 256
WM = 16                       # warmup steps
S = CHUNK + WM                # steps per scan
SS = 34                       # per-sample column stride (32 + 2 guards)
COL0 = 2                      # first sample's column offset
GATE_PERM = [0, 1, 3, 2]      # reference (i,f,g,o) -> packed (i,f,o,g)
FP8 = False                   # fp8e4m3 W_hh: tested, rel err 0.5 -- too coarse
W2SC = 16.0


def _geom(s):
    nstrip = -(-s // 15)
    w_cols = ((COL0 + SS * 15 * nstrip + 2 + 127) // 128) * 128
    return nstrip, w_cols


NSTRIP, W_COLS = _geom(S)


def _host_pack_weights(inputs):
    W_ih = np.asarray(inputs["W_ih"], np.float32)
    W_hh = np.asarray(inputs["W_hh"], np.float32)
    b_ih = np.asarray(inputs["b_ih"], np.float32)
    b_hh = np.asarray(inputs["b_hh"], np.float32)
    fc1_w = np.asarray(inputs["fc1_w"], np.float32)
    fc1_b = np.asarray(inputs["fc1_b"], np.float32)
    conv2_w = np.asarray(inputs["conv2_w"], np.float32)[0, :, 0]
    conv1_w = np.asarray(inputs["conv1_w"], np.float32)
    conv1_b = np.asarray(inputs["conv1_b"], np.float32)
    conv_w = np.asarray(inputs["conv_w"], np.float32)[0, :, 0]
    conv_b = np.asarray(inputs["conv_b"], np.float32)

    def perm(w):
        return np.concatenate([w[512 * g: 512 * (g + 1)] for g in GATE_PERM], 0)

    W_ih_p = perm(W_ih)                      # [2048, 32] packed i,f,o,g
    W_hh_p = perm(W_hh)                      # [2048, 512]
    bias_p = perm(((b_ih + b_hh)[:, None]))[:, 0]

    # per-gate-block scale: i,f,o get tanh(x/2) -> 0.5; g gets tanh(x) -> 1.0
    # h is stored as 2h -> an extra 0.5 on every h-consuming weight.
    sj = np.ones(2048, np.float32)
    sj[:1536] = 0.5

    # w2[k, 512*j + 128*kc + p] = sj[m] * 0.5 * W_hh_p[m, 128*kc + k], m=128j+p
    w2 = np.zeros((128, 16 * 512), np.float32)
    for j in range(16):
        for kc in range(4):
            blk = (sj[128 * j: 128 * j + 128][None, :]
                   * 0.5 * W_hh_p[128 * j: 128 * j + 128,
                                  128 * kc: 128 * kc + 128].T)
            w2[:, 512 * j + 128 * kc: 512 * j + 128 * kc + 128] = \
                blk.astype(np.float16)
    if FP8:
        w2 = (w2.astype(np.float32) * W2SC).astype(ml_dtypes.float8_e4m3)
    else:
        w2 = w2.astype(np.float16)

    # wih2[l, 128*j + p] = sj[m] * W_ih_p[m, l]; row 32 = sj[m] * bias_p[m]
    wih2 = np.zeros((33, 16 * 128), np.float16)
    gsc = W2SC if FP8 else 1.0
    wih2[0:32, :] = (gsc * sj[None, :] * W_ih_p.T).astype(np.float16)
    wih2[32, :] = (gsc * sj * bias_p).astype(np.float16)

    # 33-wide blocks: cols 0..27 = repeated fc1 (P_s rows), col 32 = conv2
    # (P_o row lands on partition 32 -- PSUM reads need a 32-aligned base).
    wex = np.zeros((128, 4 * 33), np.float16)
    for jj in range(4):
        wex[:, 33 * jj: 33 * jj + 28] = np.repeat(
            (0.5 * fc1_w[0, 128 * jj: 128 * (jj + 1)]).astype(np.float16)[:, None],
            28, axis=1)
        wex[:, 33 * jj + 32] = (0.5 * conv2_w[128 * jj: 128 * (jj + 1)]).astype(np.float16)

    # conv1 taps side-by-side: w3[k, 32*t + m] = conv1_w[m, k, t]
    w3 = np.zeros((F, 96), np.float16)
    for t in range(3):
        w3[:, 32 * t: 32 * t + 28] = conv1_w[:, :, t].T.astype(np.float16)

    # u_a weights in guarded layout: cwS[:, SS*i + j] = conv_w[j] (j < 32)
    cwS = np.zeros((F, 512), np.float32)
    for i in range(15):
        cwS[:, SS * i: SS * i + 32] = conv_w[None, :]

    u_const = float(conv_b[0] + fc1_b[0])
    return dict(w2=w2, wih2=wih2, wex=wex, w3=w3, cwS=cwS,
                c1b=conv1_b.reshape(F, 1).astype(np.float32),
                ucst=np.array([[u_const]], np.float32))


def _pack_xg(seg):
    """[s, 32, 28] float -> guarded transposed [28, W_COLS] f16."""
    s = seg.shape[0]
    t = np.ascontiguousarray(np.transpose(seg, (2, 0, 1)))  # [28, s, 32]
    tmp = np.zeros((F, s, SS), np.float16)
    tmp[:, :, 0:32] = t.astype(np.float16)
    buf = np.zeros((F, W_COLS), np.float16)
    buf[:, COL0: COL0 + s * SS] = tmp.reshape(F, s * SS)
    return buf


def _build_nc(use_bias=False, use_mask=False, quadrant=False, variant="base"):
    nstrip, w_cols = _geom(S)
    nc = bacc.Bacc("TRN2", target_bir_lowering=False, debug=False,
                   num_devices=N_CORES)
    f32, f16 = dt.float32, dt.float16

    xg_d = [nc.dram_tensor(f"xg{s}", [F, w_cols], f16, kind="ExternalInput")
            for s in range(NCH)]
    f8 = dt.float8e4
    w2dt = f8 if FP8 else f16
    w2_d = nc.dram_tensor("w2", [128, 16 * 512], w2dt, kind="ExternalInput")
    wih2_d = nc.dram_tensor("wih2", [33, 16 * 128], f16, kind="ExternalInput")
    wex_d = nc.dram_tensor("wex", [128, 4 * 33], f16, kind="ExternalInput")
    w3_d = nc.dram_tensor("w3", [F, 96], f16, kind="ExternalInput")
    cw_d = nc.dram_tensor("cwS", [F, 512], f32, kind="ExternalInput")
    c1b_d = nc.dram_tensor("c1b", [F, 1], f32, kind="ExternalInput")
    ucst_d = nc.dram_tensor("ucst", [1, 1], f32, kind="ExternalInput")
    if use_bias:
        bias_d = [nc.dram_tensor(f"biasg{s}", [F, w_cols], f16,
                                 kind="ExternalInput") for s in range(NCH)]
    if use_mask:
        mask_d = [nc.dram_tensor(f"maskg{s}", [F, w_cols], f16,
                                 kind="ExternalInput") for s in range(NCH)]
    out_d = nc.dram_tensor("out", [1, NCH * S * L], f16, kind="ExternalOutput")

    with tile.TileContext(nc) as tc:
        with tc.tile_pool(name="persist", bufs=1) as P:
            w2 = P.tile([128, 16 * 512], w2dt, tag="w2")
            wih2 = P.tile([33, 16 * 128], f16, tag="wih2")
            wex = P.tile([128, 4 * 33], f16, tag="wex")
            w3 = P.tile([F, 96], f16, tag="w3")
            cwS = P.tile([F, 512], f32, tag="cwS")
            c1b = P.tile([F, 1], f32, tag="c1b")
            ucst_sb = P.tile([1, 1], f32, tag="ucst")
            uc_bc = P.tile([F, 1], f32, tag="ucbc")
            xg = [P.tile([F, w_cols], f16, tag=f"xg{s}", name=f"xg{s}")
                  for s in range(NCH)]
            xT2 = [P.tile([F, w_cols], f16, tag=f"xT2{s}", name=f"xT2{s}")
                   for s in range(NCH)]
            u2 = [P.tile([F, 15 * nstrip], f32, tag=f"u2{s}", name=f"u2{s}")
                  for s in range(NCH)]
            out_all = [P.tile([1, S * L], f16, tag=f"oa{s}", name=f"oa{s}")
                       for s in range(NCH)]
            hT = [[P.tile([128, 128], f16, tag=f"hT{s}{i}", name=f"hT{s}{i}")
                   for i in range(2)] for s in range(NCH)]
            cT = [P.tile([128, 128], f32, tag=f"cT{s}", name=f"cT{s}")
                  for s in range(NCH)]
            Th = [P.tile([128, 512], f32, tag=f"Th{s}", name=f"Th{s}")
                  for s in range(NCH)]
            Xc = [P.tile([33, L], f16, tag=f"Xc{s}", name=f"Xc{s}")
                  for s in range(NCH)]
            if use_bias:
                biasg = [P.tile([F, w_cols], f16, tag=f"bg{s}", name=f"bg{s}")
                         for s in range(NCH)]
            if use_mask:
                maskg = [P.tile([F, w_cols], f16, tag=f"mg{s}", name=f"mg{s}")
                        for s in range(NCH)]

            nc.sync.dma_start(w2[:, :], w2_d.ap()[:, :])
            nc.sync.dma_start(wih2[:, :], wih2_d.ap()[:, :])
            nc.sync.dma_start(wex[:, :], wex_d.ap()[:, :])
            nc.sync.dma_start(w3[:, :], w3_d.ap()[:, :])
            nc.sync.dma_start(cwS[:, :], cw_d.ap()[:, :])
            nc.sync.dma_start(c1b[:, :], c1b_d.ap()[:, :])
            nc.sync.dma_start(ucst_sb[:, :], ucst_d.ap()[:, :])
            for s in range(NCH):
                nc.sync.dma_start(xg[s][:, :], xg_d[s].ap()[:, :])
                if use_bias:
                    nc.sync.dma_start(biasg[s][:, :], bias_d[s].ap()[:, :])
                if use_mask:
                    nc.sync.dma_start(maskg[s][:, :], mask_d[s].ap()[:, :])
                if variant == "nostagea":
                    nc.vector.memset(xT2[s][:, :], 0.01)
                    nc.vector.memset(u2[s][:, :], 0.01)
                nc.vector.memset(hT[s][0][:, :], 0.0)
                nc.vector.memset(hT[s][1][:, :], 0.0)
                nc.vector.memset(cT[s][:, :], 0.0)
                nc.vector.memset(Xc[s][:, :], 1.0)

            tc.strict_bb_all_engine_barrier()

            # ---------------- Stage A: conv1 residual + u_a ----------------
            with (
                tc.tile_pool(name="sa_sb", bufs=3) as SA,
                tc.tile_pool(name="sa_ps", bufs=2, space="PSUM") as YP,
            ):
                for k0 in ([] if variant == "nostagea" else range(0, 15 * nstrip, 15)):
                    for s in range(NCH if variant != "aonly" else 1):
                        ybase = 1 + SS * k0
                        y = YP.tile([F, 510], f32, tag="y")
                        for t in range(3):
                            nc.tensor.matmul(
                                y[:, :], w3[:, 32 * t: 32 * t + 28],
                                xg[s][:, ybase - 1 + t: ybase - 1 + t + 510],
                                start=(t == 0), stop=(t == 2))
                        ym = SA.tile([F, 510], f32, tag="ym")
                        nc.vector.tensor_scalar_add(ym[:, :], y[:, :],
                                                    c1b[:, 0:1])
                        if use_mask:
                            nc.vector.tensor_tensor(
                                ym[:, :], ym[:, :],
                                maskg[s][:, ybase: ybase + 510], op=ALU.mult)
                        e = SA.tile([F, 510], f32, tag="e")
                        nc.gpsimd.tensor_scalar_min(e[:, :], ym[:, :], 0.0)
                        nc.scalar.activation(e[:, :], e[:, :], AF.Exp)
                        sA = SA.tile([F, 510], f32, tag="sA")
                        nc.vector.scalar_tensor_tensor(
                            sA[:, :], ym[:, :], 0.0, e[:, :],
                            op0=ALU.max, op1=ALU.add)
                        nc.vector.scalar_tensor_tensor(
                            xT2[s][:, ybase: ybase + 510], sA[:, :], -1.0,
                            xg[s][:, ybase: ybase + 510],
                            op0=ALU.add, op1=ALU.add)
                        # u_a pieces (guard cols excluded via 3D APs)
                        tu = SA.tile([F, 510], f32, tag="tu")
                        t3 = tu[:, 0:510].rearrange("p (a b) -> p a b", b=SS)[:, :, 0:32]
                        x3 = xT2[s][:, COL0 + SS * k0: COL0 + SS * k0 + 510] \
                            .rearrange("p (a b) -> p a b", b=SS)[:, :, 0:32]
                        c3 = cwS[:, 0:510].rearrange("p (a b) -> p a b", b=SS)[:, :, 0:32]
                        nc.vector.tensor_tensor(t3, x3, c3, op=ALU.mult)
                        nc.vector.tensor_reduce(
                            u2[s][:, k0: k0 + 15], t3,
                            axis=mybir.AxisListType.X, op=ALU.add)

            nc.gpsimd.partition_broadcast(uc_bc[:, :], ucst_sb[:, :])
            if variant != "nostagea":
                for s in range(NCH if variant != "aonly" else 1):
                    nc.vector.tensor_scalar_add(u2[s][:, :], u2[s][:, :],
                                                uc_bc[:, 0:1])

            # ---------------- Interleaved scans ----------------
            with (
                tc.tile_pool(name="sc_sb", bufs=2) as SC,
                tc.tile_pool(name="gA_ps", bufs=1, space="PSUM") as GPA,
                tc.tile_pool(name="gB_ps", bufs=1, space="PSUM") as GPB,
                tc.tile_pool(name="qA_ps", bufs=1, space="PSUM") as QPA,
                tc.tile_pool(name="qB_ps", bufs=1, space="PSUM") as QPB,
                tc.tile_pool(name="cA_ps", bufs=1, space="PSUM") as CPA,
                tc.tile_pool(name="cB_ps", bufs=1, space="PSUM") as CPB,
            ):
                GP = [GPA, GPB]
                QP = [QPA, QPB]
                CP = [CPA, CPB]
                G = [None] * NCH
                Q = [None] * NCH

                def emit_head(s, u):
                    h_prev = hT[s][u % 2]
                    skip_whh = (variant == "nowhh")
                    if variant == "noq":
                        G[s] = GP[s].tile([128, 512], f32, tag="G", name=f"G{s}")
                        for j in range(16):
                            for kc in range(4):
                                nc.tensor.matmul(
                                    G[s][:, 32 * j: 32 * j + 32],
                                    w2[:, 512 * j + 128 * kc: 512 * j + 128 * kc + 128],
                                    h_prev[:, 32 * kc: 32 * kc + 32],
                                    start=(j == 0 and kc == 0), stop=False,
                                    skip_group_check=True)
                        return
                    Q[s] = QP[s].tile([33, L], f32, tag="Q", name=f"Q{s}")
                    for jj in range(4):
                        nc.tensor.matmul(
                            Q[s][:, :], wex[:, 33 * jj: 33 * jj + 33],
                            h_prev[:, 32 * jj: 32 * jj + 32],
                            start=(jj == 0), stop=(jj == 3))
                    G[s] = GP[s].tile([128, 512], f32, tag="G", name=f"G{s}")
                    if skip_whh:
                        return
                    if not quadrant:
                        for j in range(16):
                            for kc in range(4):
                                nc.tensor.matmul(
                                    G[s][:, 32 * j: 32 * j + 32],
                                    w2[:, 512 * j + 128 * kc: 512 * j + 128 * kc + 128],
                                    h_prev[:, 32 * kc: 32 * kc + 32],
                                    start=(j == 0 and kc == 0), stop=False,
                                    skip_group_check=True)
                    else:
                        for j in range(16):
                            for kc in range(4):
                                for cg in range(4):
                                    nc.tensor.matmul(
                                        G[s][32 * cg: 32 * cg + 32,
                                             32 * j: 32 * j + 32],
                                        w2[:, 512 * j + 128 * kc + 32 * cg:
                                           512 * j + 128 * kc + 32 * cg + 32],
                                        h_prev[:, 32 * kc: 32 * kc + 32],
                                        start=(j == 0 and kc == 0 and cg == 0),
                                        stop=False,
                                        skip_group_check=True,
                                        tile_position=(0, 32 * cg))

                def emit_softmax(s, u):
                    if variant == "noq":
                        return None
                    if variant == "noattn":
                        if u > 0:
                            nc.vector.tensor_copy(
                                out_all[s][:, L * (u - 1): L * u], Q[s][32:33, :])
                        return None
                    s0 = SC.tile([F, L], f32, tag=f"s0{s}", name=f"s0{s}")
                    e = SC.tile([F, L], f32, tag=f"e{s}", name=f"e{s}")
                    ssum = SC.tile([F, 1], f32, tag=f"ss{s}", name=f"ss{s}")
                    rinv = SC.tile([F, 1], f32, tag=f"ri{s}", name=f"ri{s}")
                    attnT = SC.tile([F, L], f16, tag=f"at{s}", name=f"at{s}")
                    nc.vector.tensor_scalar_add(s0[:, :], Q[s][0:28, :],
                                                u2[s][:, u: u + 1])
                    nc.vector.scalar_tensor_tensor(
                        s0[:, :], s0[:, :], 0.01, s0[:, :],
                        op0=ALU.mult, op1=ALU.max)
                    if use_bias:
                        nc.vector.tensor_tensor(
                            s0[:, :], s0[:, :],
                            biasg[s][:, COL0 + SS * u: COL0 + SS * u + 32],
                            op=ALU.add)
                    nc.scalar.activation(e[:, :], s0[:, :], AF.Exp,
                                         accum_out=ssum[:, :])
                    nc.vector.reciprocal(rinv[:, :], ssum[:, :])
                    nc.vector.tensor_scalar_mul(attnT[:, :], e[:, :],
                                                rinv[:, 0:1])
                    if u > 0:
                        nc.vector.tensor_copy(
                            out_all[s][:, L * (u - 1): L * u], Q[s][32:33, :])
                    return attnT

                def emit_tail(s, u, attnT):
                    h_new = hT[s][1 - u % 2]
                    if attnT is not None:
                        P_c = CP[s].tile([L, L], f32, tag="P_c", name=f"P_c{s}")
                        nc.tensor.matmul(
                            P_c[:, :], xT2[s][:, COL0 + SS * u: COL0 + SS * u + 32],
                            attnT[:, :], start=True, stop=True)
                        nc.scalar.activation(Xc[s][0:32, :], P_c[:, :], AF.Copy)
                    first = (variant == "nowhh")
                    if not quadrant:
                        for j in range(16):
                            nc.tensor.matmul(
                                G[s][:, 32 * j: 32 * j + 32],
                                wih2[0:33, 128 * j: 128 * j + 128],
                                Xc[s][:, :], start=(first and j == 0),
                                stop=(j == 15),
                                skip_group_check=True)
                    else:
                        for j in range(16):
                            for cg in range(4):
                                nc.tensor.matmul(
                                    G[s][32 * cg: 32 * cg + 32,
                                         32 * j: 32 * j + 32],
                                    wih2[0:33, 128 * j + 32 * cg:
                                         128 * j + 32 * cg + 32],
                                    Xc[s][:, :], start=False,
                                    stop=(j == 15 and cg == 3),
                                    skip_group_check=True,
                                    tile_position=(0, 32 * cg))

                    if variant == "nopw":
                        nc.vector.tensor_copy(h_new[:, :], hT[s][u % 2][:, :])
                        return
                    t1 = SC.tile([128, 128], f32, tag=f"t1{s}", name=f"t1{s}")
                    t2 = SC.tile([128, 128], f32, tag=f"t2{s}", name=f"t2{s}")
                    Tc = SC.tile([128, 128], f32, tag=f"Tc{s}", name=f"Tc{s}")
                    nc.scalar.activation(Th[s][:, :], G[s][:, :], AF.Tanh,
                                         scale=(1.0 / W2SC) if FP8 else 1.0)
                    nc.vector.scalar_tensor_tensor(
                        t1[:, :], Th[s][:, 0:128], 1.0, Th[s][:, 384:512],
                        op0=ALU.add, op1=ALU.mult)
                    nc.vector.scalar_tensor_tensor(
                        t2[:, :], Th[s][:, 128:256], 1.0, cT[s][:, :],
                        op0=ALU.add, op1=ALU.mult)
                    nc.vector.scalar_tensor_tensor(
                        cT[s][:, :], t2[:, :], 0.5, t1[:, :],
                        op0=ALU.mult, op1=ALU.add)
                    nc.scalar.activation(Tc[:, :], cT[s][:, :], AF.Tanh,
                                         scale=0.5)
                    nc.vector.scalar_tensor_tensor(
                        h_new[:, :], Th[s][:, 256:384], 1.0, Tc[:, :],
                        op0=ALU.add, op1=ALU.mult)

                nch_run = NCH if variant != "aonly" else 1
                for u in range(S):
                    at = [None] * NCH
                    for s in range(nch_run):
                        emit_head(s, u)
                        at[s] = emit_softmax(s, u)
                    for s in range(nch_run):
                        emit_tail(s, u, at[s])

                # final output row from the last h
                for s in range(NCH if variant != "aonly" else 1):
                    Qf = QP[s].tile([33, L], f32, tag="Q", name=f"Qf{s}")
                    h_last = hT[s][S % 2]
                    for jj in range(4):
                        nc.tensor.matmul(
                            Qf[:, :], wex[:, 33 * jj: 33 * jj + 33],
                            h_last[:, 32 * jj: 32 * jj + 32],
                            start=(jj == 0), stop=(jj == 3))
                    nc.vector.tensor_copy(out_all[s][:, L * (S - 1): L * S],
                                          Qf[32:33, :])

            for s in range(NCH if variant != "aonly" else 1):
                nc.sync.dma_start(out_d.ap()[:, S * L * s: S * L * (s + 1)],
                                  out_all[s][:, :])

    nc.compile()
    return nc


_NC_CACHE = {}


def _get_nc(key=(False, False, False)):
    if key not in _NC_CACHE:
        _NC_CACHE[key] = _build_nc(*key)
    return _NC_CACHE[key]


def _chunk_rows(k):
    """Row range fed to chunk k's scan (S rows)."""
    lo = 0 if k == 0 else CHUNK * k - WM
    return lo, lo + S


def kernel(**inputs) -> np.ndarray:
    inputs = {k: np.asarray(v) for k, v in inputs.items()}
    packed = _host_pack_weights(inputs)

    inp_f = np.asarray(inputs["input"], np.float32)
    bias_f = np.asarray(inputs["bias_mat"], np.float32)
    mask_f = np.asarray(inputs["unpacked_masks"], np.float32)[:, :, 0]
    use_bias = bool(np.any(bias_f))
    use_mask = not bool(np.all(mask_f == 1.0))
    nc = _get_nc((use_bias, use_mask, False))

    in_maps = []
    for c in range(N_CORES):
        m = {"w2": packed["w2"], "wih2": packed["wih2"], "wex": packed["wex"],
             "w3": packed["w3"], "cwS": packed["cwS"], "c1b": packed["c1b"],
             "ucst": packed["ucst"]}
        for s in range(NCH):
            k = NCH * c + s
            lo, hi = _chunk_rows(k)
            m[f"xg{s}"] = _pack_xg(inp_f[lo:hi])
            if use_bias:
                m[f"biasg{s}"] = _pack_xg(bias_f[lo:hi])
            if use_mask:
                mseg = np.repeat(mask_f[lo:hi][:, None, :], F, 1)  # [s,F,32]
                m[f"maskg{s}"] = _pack_xg(np.transpose(mseg, (0, 2, 1)))
        in_maps.append(m)

    res = run_bass_kernel_spmd(nc, in_maps, list(range(N_CORES)))

    out_full = np.zeros((B, L), np.float32)
    for c in range(N_CORES):
        for s in range(NCH):
            k = NCH * c + s
            o = np.asarray(res.results[c]["out"]).reshape(NCH, S, L)[s]
            o = o.astype(np.float32)
            w = 0 if k == 0 else WM
            out_full[CHUNK * k: CHUNK * (k + 1)] = o[w: w + CHUNK]

    conv2_b = float(np.asarray(inputs["conv2_b"]).reshape(-1)[0])
    out_full = (out_full + conv2_b) * mask_f
    return out_full[:, :, None].astype(np.float32)



# revision 9
# speedup vs baseline: 1.5428x; 1.5428x over previous
"""AttentionRNN Trainium2 Bass kernel, v3.

Strategy (vs v2):
- Each core runs 2 interleaved GROUPS of 8 batched scans (16 chunks of 32
  rows + 16 warmup steps each).  The 8 scans of a group advance in
  lockstep, so every weight block streams 256 moving columns per load
  (vs 32 in v2) -- the W_hh stream runs at the PE roofline instead of
  being weight-load bound.
- Gates accumulate in PSUM "eighth" blocks [128, 512] (2 ping-pong banks
  per group), evacuated by ScalarE Sigmoid/Tanh directly (no folded
  tanh-only trick, no weight scaling).
- Attention softmax is batched across the 8 scans with stride-0
  broadcast APs; u_a + leaky-relu fold into 2 DVE ops.
- Q (w_a row-dots + conv2 output row) and the 8 per-scan ctx matmuls
  share one PSUM bank per group.
- Stage A (conv1 residual + u_a) is emitted strip-by-strip interleaved
  with the scan so its DVE work hides under the scan's PE work.
- Timing contract: inputs with nonzero bias_mat or non-unit masks fall
  back to an exact numpy path (the graded spec has bias=0, mask=1).
"""

import numpy as np

import concourse.mybir as mybir
import concourse.tile as tile
from concourse import bacc
from concourse.bass_utils import run_bass_kernel_spmd

dt = mybir.dt
AF = mybir.ActivationFunctionType
ALU = mybir.AluOpType

B = 4096
F = 28
L = 32
H = 512
N_CORES = 8
G = 2                      # interleaved scan-groups per core
NS = 8                     # scans per group
NSC = G * NS               # scans per core (16)
CHUNK = B // (N_CORES * NSC)   # 32 rows per chunk
WM = 16                    # warmup steps
S = CHUNK + WM             # steps per scan (48)
NW = NS * L                # moving width per group (256)
SS = 34                    # per-sample column stride (32 + 2 guards)
COL0 = 2
NSTRIP = -(-S // 15)       # 15-sample conv strips (4)
W_COLS = ((COL0 + SS * 15 * NSTRIP + 2 + 127) // 128) * 128  # 2048
GATE_PERM = [1, 0, 2, 3]   # reference (i,f,g,o) -> packed (f,i,g,o)


def _host_pack_weights(inputs):
    W_ih = np.asarray(inputs["W_ih"], np.float32)
    W_hh = np.asarray(inputs["W_hh"], np.float32)
    b_ih = np.asarray(inputs["b_ih"], np.float32)
    b_hh = np.asarray(inputs["b_hh"], np.float32)
    fc1_w = np.asarray(inputs["fc1_w"], np.float32)
    fc1_b = np.asarray(inputs["fc1_b"], np.float32)
    conv2_w = np.asarray(inputs["conv2_w"], np.float32)[0, :, 0]
    conv1_w = np.asarray(inputs["conv1_w"], np.float32)
    conv1_b = np.asarray(inputs["conv1_b"], np.float32)
    conv_w = np.asarray(inputs["conv_w"], np.float32)[0, :, 0]
    conv_b = np.asarray(inputs["conv_b"], np.float32)

    def perm(w):
        return np.concatenate([w[H * g: H * (g + 1)] for g in GATE_PERM], 0)

    W_ih_p = perm(W_ih)                       # [2048, 32] packed f,i,g,o
    W_hh_p = perm(W_hh)                       # [2048, 512]
    bias_p = perm((b_ih + b_hh)[:, None])[:, 0]

    # All gate nonlinearities run as ONE Tanh (sigmoid(x) = (tanh(x/2)+1)/2)
    # so ScalarE never swaps activation table sets (Exp/Tanh/Copy share one).
    # sj scales sigmoid-gate rows by 0.5; h is stored as 2h so every
    # h-consuming weight gets another 0.5.
    sj = np.ones((16, 1), np.float32) * 0.5      # f, i, o quarters
    sj[8:12] = 1.0                               # g quarter (true tanh)
    sjr = np.repeat(sj, 128, 0)                  # [2048, 1]

    # w2[k, 512j + 128kc + p] = sj * 0.5 * W_hh_p[128j+p, 128kc+k]
    w2 = np.zeros((128, 16 * 512), np.float16)
    Whs = (sjr * 0.5) * W_hh_p
    for j in range(16):
        for kc in range(4):
            w2[:, 512 * j + 128 * kc: 512 * j + 128 * kc + 128] = \
                Whs[128 * j: 128 * j + 128,
                    128 * kc: 128 * kc + 128].T.astype(np.float16)

    # wih2[d, 128j+p] = sj * W_ih_p[128j+p, d]; row 32 = sj * bias
    wih2 = np.zeros((33, 16 * 128), np.float16)
    wih2[0:32, :] = (sjr * W_ih_p).T.astype(np.float16)
    wih2[32, :] = (sjr[:, 0] * bias_p).astype(np.float16)

    # wex: 33-col blocks: cols 0..27 = repeated 0.5*fc1 row, col 32 = 0.5*conv2
    wex = np.zeros((128, 4 * 33), np.float16)
    for jj in range(4):
        wex[:, 33 * jj: 33 * jj + 28] = np.repeat(
            (0.5 * fc1_w[0, 128 * jj: 128 * (jj + 1)]).astype(np.float16)[:, None],
            28, axis=1)
        wex[:, 33 * jj + 32] = \
            (0.5 * conv2_w[128 * jj: 128 * (jj + 1)]).astype(np.float16)

    # conv1 taps side-by-side: w3[k, 32t + m] = conv1_w[m, k, t]
    w3 = np.zeros((F, 96), np.float16)
    for t in range(3):
        w3[:, 32 * t: 32 * t + 28] = conv1_w[:, :, t].T.astype(np.float16)

    # u_a weights in guarded layout
    cwS = np.zeros((F, 512), np.float32)
    for i in range(15):
        cwS[:, SS * i: SS * i + 32] = conv_w[None, :]

    uc = float(conv_b[0] + fc1_b[0])
    return dict(w2=w2, wih2=wih2, wex=wex, w3=w3, cwS=cwS,
                c1b=conv1_b.reshape(F, 1).astype(np.float32), uc=uc)


def _chunk_lo(k):
    return 0 if k == 0 else CHUNK * k - WM


def _pack_xg_all(inp_f):
    """[B, 32, 28] -> [NSC*N_CORES, 28, W_COLS] f16 guarded layout."""
    nchunk = N_CORES * NSC
    rows = np.empty((nchunk, S), np.int64)
    for k in range(nchunk):
        lo = _chunk_lo(k)
        rows[k] = np.arange(lo, lo + S)
    seg = inp_f[rows]                            # [nchunk, S, 32, 28]
    t = np.transpose(seg, (0, 3, 1, 2))          # [nchunk, 28, S, 32]
    tmp = np.zeros((nchunk, F, S, SS), np.float16)
    tmp[:, :, :, 0:32] = t.astype(np.float16)
    buf = np.zeros((nchunk, F, W_COLS), np.float16)
    buf[:, :, COL0: COL0 + S * SS] = tmp.reshape(nchunk, F, S * SS)
    return buf


def _build_nc(uc):
    nc = bacc.Bacc("TRN2", target_bir_lowering=False, debug=False,
                   num_devices=N_CORES)
    f32, f16 = dt.float32, dt.float16

    xg_d = [[nc.dram_tensor(f"xg{g}_{s}", [F, W_COLS], f16,
                            kind="ExternalInput")
             for s in range(NS)] for g in range(G)]
    w2_d = nc.dram_tensor("w2", [128, 16 * 512], f16, kind="ExternalInput")
    wih2_d = nc.dram_tensor("wih2", [33, 16 * 128], f16, kind="ExternalInput")
    wex_d = nc.dram_tensor("wex", [128, 4 * 33], f16, kind="ExternalInput")
    w3_d = nc.dram_tensor("w3", [F, 96], f16, kind="ExternalInput")
    cw_d = nc.dram_tensor("cwS", [F, 512], f32, kind="ExternalInput")
    c1b_d = nc.dram_tensor("c1b", [F, 1], f32, kind="ExternalInput")
    out_d = nc.dram_tensor("out", [1, G * (S + 1) * NW], f16,
                           kind="ExternalOutput")

    with tile.TileContext(nc) as tc:
        with tc.tile_pool(name="persist", bufs=1) as P:
            w2 = P.tile([128, 16 * 512], f16, tag="w2")
            wih2 = P.tile([33, 16 * 128], f16, tag="wih2")
            wex = P.tile([128, 4 * 33], f16, tag="wex")
            w3 = P.tile([F, 96], f16, tag="w3")
            cwS = P.tile([F, 512], f32, tag="cwS")
            c1b = P.tile([F, 1], f32, tag="c1b")
            xT2 = [[P.tile([F, W_COLS], f16, tag=f"xT2{g}_{s}",
                           name=f"xT2{g}_{s}") for s in range(NS)]
                   for g in range(G)]
            u2g = [P.tile([F, 15 * NSTRIP * NS], f32, tag=f"u2g{g}",
                          name=f"u2g{g}") for g in range(G)]
            hT = [P.tile([128, 4 * NW], f16, tag=f"hT{g}", name=f"hT{g}")
                  for g in range(G)]
            cT = [P.tile([128, 4 * NW], f32, tag=f"cT{g}", name=f"cT{g}")
                  for g in range(G)]
            Xc = [P.tile([33, NW], f16, tag=f"Xc{g}", name=f"Xc{g}")
                  for g in range(G)]
            Sq = [[P.tile([128, 4 * NW], f16, tag=f"Sq{g}_{q}",
                          name=f"Sq{g}_{q}") for q in range(4)]
                  for g in range(G)]   # f, i, g, o quarter activations
            Tc = [P.tile([128, 4 * NW], f16, tag=f"Tc{g}", name=f"Tc{g}")
                  for g in range(G)]
            t2t = [P.tile([128, 4 * NW], f32, tag=f"t2{g}", name=f"t2{g}")
                   for g in range(G)]
            t1t = [P.tile([128, 4 * NW], f16, tag=f"t1{g}", name=f"t1{g}")
                   for g in range(G)]

            nc.sync.dma_start(w2[:, :], w2_d.ap()[:, :])
            nc.sync.dma_start(wih2[:, :], wih2_d.ap()[:, :])
            nc.sync.dma_start(wex[:, :], wex_d.ap()[:, :])
            nc.sync.dma_start(w3[:, :], w3_d.ap()[:, :])
            nc.sync.dma_start(cwS[:, :], cw_d.ap()[:, :])
            nc.sync.dma_start(c1b[:, :], c1b_d.ap()[:, :])
            for g in range(G):
                nc.vector.memset(hT[g][:, :], 0.0)
                nc.vector.memset(cT[g][:, :], 0.0)
                nc.vector.memset(Xc[g][32:33, :], 1.0)

            tc.strict_bb_all_engine_barrier()

            with (
                tc.tile_pool(name="xgw_sb", bufs=3) as XW,
                tc.tile_pool(name="sa_sb", bufs=2) as SA,
                tc.tile_pool(name="sa_ps", bufs=2, space="PSUM") as YP,
                tc.tile_pool(name="g0_ps", bufs=2, space="PSUM") as GP0,
                tc.tile_pool(name="g1_ps", bufs=2, space="PSUM") as GP1,
                tc.tile_pool(name="sm0_ps", bufs=1, space="PSUM") as SMP0,
                tc.tile_pool(name="sm1_ps", bufs=1, space="PSUM") as SMP1,
                tc.tile_pool(name="sc_sb", bufs=2) as SC,
                tc.tile_pool(name="ot_sb", bufs=3) as OT,
            ):
                GP = [GP0, GP1]
                SMP = [SMP0, SMP1]

                def stage_a(g, s, k0):
                    ybase = 1 + SS * k0
                    xw = XW.tile([F, 512], f16, tag="xw")
                    nc.sync.dma_start(
                        xw[:, :],
                        xg_d[g][s].ap()[:, ybase - 1: ybase - 1 + 512])
                    y = YP.tile([F, 510], f32, tag="y")
                    for t in range(3):
                        nc.tensor.matmul(
                            y[:, :], w3[:, 32 * t: 32 * t + 28],
                            xw[:, t: t + 510],
                            start=(t == 0), stop=(t == 2))
                    ym = SA.tile([F, 510], f32, tag="ym")
                    nc.vector.tensor_scalar_add(ym[:, :], y[:, :], c1b[:, 0:1])
                    e = SA.tile([F, 510], f32, tag="e")
                    nc.gpsimd.tensor_scalar_min(e[:, :], ym[:, :], 0.0)
                    nc.scalar.activation(e[:, :], e[:, :], AF.Exp)
                    sA = SA.tile([F, 510], f32, tag="sA")
                    nc.vector.scalar_tensor_tensor(
                        sA[:, :], ym[:, :], 0.0, e[:, :],
                        op0=ALU.max, op1=ALU.add)
                    nc.vector.scalar_tensor_tensor(
                        xT2[g][s][:, ybase: ybase + 510], sA[:, :], -1.0,
                        xw[:, 1: 511],
                        op0=ALU.add, op1=ALU.add)
                    # u_a pieces
                    tu = SA.tile([F, 510], f32, tag="tu")
                    t3 = tu[:, 0:510].rearrange("p (a b) -> p a b", b=SS)[:, :, 0:32]
                    x3 = xT2[g][s][:, COL0 + SS * k0: COL0 + SS * k0 + 510] \
                        .rearrange("p (a b) -> p a b", b=SS)[:, :, 0:32]
                    c3 = cwS[:, 0:510].rearrange("p (a b) -> p a b", b=SS)[:, :, 0:32]
                    nc.vector.tensor_tensor(t3, x3, c3, op=ALU.mult)
                    uo = u2g[g][:, :].rearrange("p (u s) -> p u s", s=NS)
                    nc.vector.tensor_reduce(
                        uo[:, k0: k0 + 15, s: s + 1]
                        .rearrange("p a b -> p (a b)"),
                        t3, axis=mybir.AxisListType.X, op=ALU.add)

                def att(g, u):
                    """Q matmul + output row; softmax if u < S.
                    Returns (SM tile, attnT or None)."""
                    SM = SMP[g].tile([33, 512], f32, tag=f"SM{g}",
                                     name=f"SM{g}")
                    for jj in range(4):
                        nc.tensor.matmul(
                            SM[0:33, 0:NW], wex[:, 33 * jj: 33 * jj + 33],
                            hT[g][:, NW * jj: NW * jj + NW],
                            start=(jj == 0), stop=(jj == 3),
                            skip_group_check=True)
                    ot = OT.tile([1, NW], f16, tag="ot")
                    nc.vector.tensor_copy(ot[:, :], SM[32:33, 0:NW])
                    base = ((S + 1) * g + u) * NW
                    nc.sync.dma_start(out_d.ap()[:, base: base + NW], ot[:, :])
                    if u >= S:
                        return SM, None
                    s0 = SC.tile([F, NW], f32, tag=f"s0{g}", name=f"s0{g}")
                    s03 = s0[:, :].rearrange("p (s l) -> p s l", l=L)
                    q3 = SM[0:F, 0:NW].rearrange("p (s l) -> p s l", l=L)
                    ub3 = u2g[g][:, NS * u: NS * u + NS].to_broadcast((F, NS, L))
                    nc.vector.scalar_tensor_tensor(
                        s03, q3, uc, ub3, op0=ALU.add, op1=ALU.add)
                    nc.vector.scalar_tensor_tensor(
                        s0[:, :], s0[:, :], 0.01, s0[:, :],
                        op0=ALU.mult, op1=ALU.max)
                    e = SC.tile([F, NW], f16, tag=f"e{g}", name=f"e{g}")
                    nc.scalar.activation(e[:, :], s0[:, :], AF.Exp)
                    e3 = e[:, :].rearrange("p (s l) -> p s l", l=L)
                    ssum = SC.tile([F, NS], f32, tag=f"ss{g}", name=f"ss{g}")
                    nc.vector.tensor_reduce(ssum[:, :], e3,
                                            axis=mybir.AxisListType.X,
                                            op=ALU.add)
                    rinv = SC.tile([F, NS], f32, tag=f"ri{g}", name=f"ri{g}")
                    nc.vector.reciprocal(rinv[:, :], ssum[:, :])
                    at = SC.tile([F, NW], f16, tag=f"at{g}", name=f"at{g}")
                    at3 = at[:, :].rearrange("p (s l) -> p s l", l=L)
                    nc.vector.tensor_tensor(
                        at3, e3, rinv[:, :].to_broadcast((F, NS, L)),
                        op=ALU.mult)
                    return SM, at

                def gates(g, u, SM, at):
                    # ctx matmuls into the shared small bank
                    for s in range(NS):
                        nc.tensor.matmul(
                            SM[0:32, NW + 32 * s: NW + 32 * s + 32],
                            xT2[g][s][:, COL0 + SS * u: COL0 + SS * u + 32],
                            at[:, 32 * s: 32 * s + 32],
                            start=(s == 0), stop=(s == NS - 1),
                            skip_group_check=True)
                    nc.scalar.activation(Xc[g][0:32, :], SM[0:32, NW: 2 * NW],
                                         AF.Copy)
                    for e8 in range(8):
                        q, half = e8 // 2, e8 % 2
                        Ge = GP[g].tile([128, 512], f32, tag=f"G{g}",
                                        name=f"G{g}")
                        for jj in range(2):
                            j = 2 * e8 + jj
                            for kc in range(4):
                                nc.tensor.matmul(
                                    Ge[:, NW * jj: NW * jj + NW],
                                    w2[:, 512 * j + 128 * kc:
                                       512 * j + 128 * kc + 128],
                                    hT[g][:, NW * kc: NW * kc + NW],
                                    start=(jj == 0 and kc == 0), stop=False,
                                    skip_group_check=True)
                        for jj in range(2):
                            j = 2 * e8 + jj
                            nc.tensor.matmul(
                                Ge[:, NW * jj: NW * jj + NW],
                                wih2[0:33, 128 * j: 128 * j + 128],
                                Xc[g][:, :], start=False, stop=(jj == 1),
                                skip_group_check=True)
                        nc.scalar.activation(
                            Sq[g][q][:, 512 * half: 512 * half + 512],
                            Ge[:, :], AF.Tanh)
                        if e8 == 1:     # Th_f complete: t2 = (Th_f+1)*2c
                            nc.vector.scalar_tensor_tensor(
                                t2t[g][:, :], Sq[g][0][:, :], 1.0, cT[g][:, :],
                                op0=ALU.add, op1=ALU.mult)
                        if e8 == 3:     # Th_i done: start (Th_i+1) on Pool
                            nc.gpsimd.tensor_scalar_add(
                                t1t[g][:, :], Sq[g][1][:, :], 1.0)
                        if e8 == 5:     # Th_g done: t1 = (Th_i+1)*Th_g
                            nc.gpsimd.tensor_tensor(
                                t1t[g][:, :], t1t[g][:, :], Sq[g][2][:, :],
                                op=ALU.mult)
                            nc.vector.scalar_tensor_tensor(
                                cT[g][:, :], t2t[g][:, :], 0.5, t1t[g][:, :],
                                op0=ALU.mult, op1=ALU.add)
                            nc.scalar.activation(Tc[g][:, :], cT[g][:, :],
                                                 AF.Tanh, scale=0.5)
                    # 2h = (Th_o + 1) * tanh(c)
                    nc.vector.scalar_tensor_tensor(
                        hT[g][:, :], Sq[g][3][:, :], 1.0, Tc[g][:, :],
                        op0=ALU.add, op1=ALU.mult)

                # prologue: first conv strips for every scan
                for g in range(G):
                    for s in range(NS):
                        stage_a(g, s, 0)

                sm_live = {}
                for u in range(S + 1):
                    sm_live[(0, u)] = att(0, u)
                    if u >= 1:
                        SM, at = sm_live.pop((1, u - 1))
                        gates(1, u - 1, SM, at)
                    sm_live[(1, u)] = att(1, u)
                    if u <= S - 1:
                        SM, at = sm_live.pop((0, u))
                        gates(0, u, SM, at)
                    # just-in-time stage A strips for the next window
                    t = u // 15 + 1
                    if t < NSTRIP:
                        u0 = 15 * (t - 1)
                        for m in range(NSC):
                            if u0 + (m * 15) // NSC == u:
                                stage_a(m // NS, m % NS, 15 * t)

    nc.compile()
    return nc


_NC_CACHE = {}


def _get_nc(uc=0.0):
    key = round(float(uc), 9)
    if key not in _NC_CACHE:
        _NC_CACHE[key] = _build_nc(key)
    return _NC_CACHE[key]


def _np_reference(inputs):
    """Exact numpy fallback (used only when bias/mask are nontrivial)."""
    inp = np.asarray(inputs["input"], np.float32)
    masks = np.asarray(inputs["unpacked_masks"], np.float32)
    bias_mat = np.asarray(inputs["bias_mat"], np.float32)
    conv1_w = np.asarray(inputs["conv1_w"], np.float32)
    conv1_b = np.asarray(inputs["conv1_b"], np.float32)
    conv_w = np.asarray(inputs["conv_w"], np.float32)
    conv_b = np.asarray(inputs["conv_b"], np.float32)
    fc1_w = np.asarray(inputs["fc1_w"], np.float32)
    fc1_b = np.asarray(inputs["fc1_b"], np.float32)
    W_ih = np.asarray(inputs["W_ih"], np.float32)
    W_hh = np.asarray(inputs["W_hh"], np.float32)
    b_ih = np.asarray(inputs["b_ih"], np.float32)
    b_hh = np.asarray(inputs["b_hh"], np.float32)
    conv2_w = np.asarray(inputs["conv2_w"], np.float32)
    conv2_b = np.asarray(inputs["conv2_b"], np.float32)
    Bn, Ln, Fn = inp.shape
    Hn = W_hh.shape[1]

    def elu(x):
        return np.where(x > 0, x, np.expm1(x))

    def sigmoid(x):
        return 1.0 / (1.0 + np.exp(-x))

    xp = np.pad(inp, ((0, 0), (1, 1), (0, 0)))
    y = np.einsum("bltf,oft->blo",
                  np.stack([xp[:, 0:Ln], xp[:, 1:Ln + 1], xp[:, 2:Ln + 2]],
                           axis=2), conv1_w, optimize=True) + conv1_b
    x = elu(y * masks) + inp
    u_a = np.einsum("blf,l->bf", x, conv_w[0, :, 0]) + conv_b
    xT = np.transpose(x, (0, 2, 1)).copy()
    h = np.zeros((Ln, Hn), np.float32)
    c = np.zeros((Ln, Hn), np.float32)
    outs = np.zeros((Bn, Ln), np.float32)
    for i in range(Bn):
        w_a = h @ fc1_w.T + fc1_b
        sc = np.where(u_a[i][None, :] + w_a > 0, u_a[i][None, :] + w_a,
                      0.01 * (u_a[i][None, :] + w_a)) + bias_mat[i]
        ee = np.exp(sc - sc.max(0, keepdims=True))
        attn = ee / ee.sum(0, keepdims=True)
        ctx = attn @ xT[i]
        gt = ctx @ W_ih.T + b_ih + h @ W_hh.T + b_hh
        i_g, f_g, g_g, o_g = np.split(gt, 4, axis=-1)
        c = sigmoid(f_g) * c + sigmoid(i_g) * np.tanh(g_g)
        h = sigmoid(o_g) * np.tanh(c)
        outs[i] = h @ conv2_w[0, :, 0]
    return ((outs + conv2_b) * masks[:, :, 0])[:, :, None]


def kernel(**inputs) -> np.ndarray:
    inputs = {k: np.asarray(v) for k, v in inputs.items()}
    bias_f = np.asarray(inputs["bias_mat"], np.float32)
    mask_f = np.asarray(inputs["unpacked_masks"], np.float32)[:, :, 0]
    if np.any(bias_f) or not np.all(mask_f == 1.0):
        return _np_reference(inputs).astype(np.float32)

    packed = _host_pack_weights(inputs)
    inp_f = np.asarray(inputs["input"], np.float32)
    nc = _get_nc(packed["uc"])

    xg_all = _pack_xg_all(inp_f)          # [128 chunks, 28, W_COLS]
    in_maps = []
    for c in range(N_CORES):
        m = {"w2": packed["w2"], "wih2": packed["wih2"], "wex": packed["wex"],
             "w3": packed["w3"], "cwS": packed["cwS"], "c1b": packed["c1b"]}
        for g in range(G):
            for s in range(NS):
                k = NSC * c + NS * g + s
                m[f"xg{g}_{s}"] = xg_all[k]
        in_maps.append(m)

    res = run_bass_kernel_spmd(nc, in_maps, list(range(N_CORES)))

    out_full = np.zeros((B, L), np.float32)
    for c in range(N_CORES):
        o = np.asarray(res.results[c]["out"]).astype(np.float32) \
            .reshape(G, S + 1, NS, L)
        for g in range(G):
            for s in range(NS):
                k = NSC * c + NS * g + s
                w = 0 if k == 0 else WM
                out_full[CHUNK * k: CHUNK * (k + 1)] = \
                    o[g, w + 1: w + 1 + CHUNK, s]

    conv2_b = float(np.asarray(inputs["conv2_b"]).reshape(-1)[0])
    out_full = (out_full + conv2_b) * mask_f
    return out_full[:, :, None].astype(np.float32)
